# revision 50
# baseline (speedup 1.0000x reference)
"""Trainium2 Bass kernel for nn_MixAttention (GAT-style mixed attention).

Strategy (8 cores, i-sharded over query rows, transposed compute):
  - Device computes scores in transposed layout [j on partitions, i free] so
    out^T += hc_chunk.T @ P^T_chunk contracts over partitions, no transposes.
  - lrelu decomposition: lrelu(x) = 0.01x + 0.99*relu(x). For both score
    terms the relu part is computed per entry; the linear part is rank-1:
    the per-i piece cancels in the row softmax, the per-j piece rides along
    as a multiplicative exp(lv[j]) folded into the mask op's scalar slot.
  - Per chunk: tA = relu(bcA + agrid_c) and tB = relu(bcB + bgrid_c) via
    single tensor_scalar(add, max-0) ops (fp16, DVE 4x mode), z = tA + tB
    (fp16 TT), exp over 4-chunk groups on ACT (scale=0.99, const bias), then
    Pm = P * explv_c * slab on GPSIMD (one STT; slab is the adjacency so
    masked entries are exactly 0).
  - PE accumulates out^T += hc_c @ Pm and rowsum += ones @ Pm in bf16.
  - Phase-0 grids (dstA, dstB, sigma, lv) are produced by tiny per-chunk
    PE matmuls into PSUM grid tiles and post-processed in batched [128, 8]
    ops, avoiding per-chunk small-op overhead.
  - All per-core constants enter via input tensors (negc0/negclv/cabA/cabB),
    so a single compiled program serves every core and every input set.
"""

import numpy as np

N = 8192
K = 256
F = 128
NC = 8
S = N // NC          # 1024 query rows per core
NCH = N // 128       # 64 j-chunks
KC = K // 128        # 2 contraction chunks
G0 = 8               # j-chunks per phase-0 stream group
W0 = G0 * 128
GRP = 4              # j-chunks per exp group / mask slab
NSLAB = NCH // GRP   # 16

_BUILD_CACHE = {}


def _build_program():
    import contextlib

    import concourse.bacc as bacc
    import concourse.tile as tile
    from concourse import mybir

    nc = bacc.Bacc("TRN2", target_bir_lowering=False, debug=False, num_devices=NC)
    dt = mybir.dt
    AF = mybir.ActivationFunctionType
    OP = mybir.AluOpType

    hctxT = nc.dram_tensor("hctxT", [K, N], dt.bfloat16, kind="ExternalInput")
    hstrT = nc.dram_tensor("hstrT", [K, N], dt.bfloat16, kind="ExternalInput")
    hctxT_my = nc.dram_tensor("hctxT_my", [K, S], dt.bfloat16,
                              kind="ExternalInput")
    hstrT_my = nc.dram_tensor("hstrT_my", [K, S], dt.bfloat16,
                              kind="ExternalInput")
    uB16 = nc.dram_tensor("uB16", [K, 3], dt.bfloat16, kind="ExternalInput")
    vA116 = nc.dram_tensor("vA116", [K, 1], dt.bfloat16, kind="ExternalInput")
    wpack = nc.dram_tensor("wpack", [K, F + 1], dt.bfloat16, kind="ExternalInput")
    maskP = nc.dram_tensor("maskP", [128, NCH * S], dt.uint16,
                           kind="ExternalInput")
    negc0 = nc.dram_tensor("negc0", [128, 1], dt.float32, kind="ExternalInput")
    negclv = nc.dram_tensor("negclv", [128, 1], dt.float32, kind="ExternalInput")
    cabA = nc.dram_tensor("cabA", [128, 1], dt.float32, kind="ExternalInput")
    cabB = nc.dram_tensor("cabB", [128, 1], dt.float32, kind="ExternalInput")
    outT = nc.dram_tensor("outT", [F, S], dt.float32, kind="ExternalOutput")

    with tile.TileContext(nc) as tc:
        with contextlib.ExitStack() as ctx:
            vecs = ctx.enter_context(tc.tile_pool(name="vecs", bufs=1))
            hcpool = ctx.enter_context(tc.tile_pool(name="hc", bufs=1))
            stp = ctx.enter_context(tc.tile_pool(name="stream", bufs=2))
            work = ctx.enter_context(tc.tile_pool(name="work", bufs=4))
            grpp = ctx.enter_context(tc.tile_pool(name="grp", bufs=3))
            pmp = ctx.enter_context(tc.tile_pool(name="pm", bufs=8))
            slabp = ctx.enter_context(tc.tile_pool(name="slabp", bufs=3))

            # ---- small inputs ----
            wpack_sb = [vecs.tile([128, F + 1], dt.bfloat16, name=f"wp{k}")
                        for k in range(KC)]
            negc0_sb = vecs.tile([128, 1], dt.float32, name="negc0_sb")
            negclv_sb = vecs.tile([128, 1], dt.float32, name="negclv_sb")
            cabA_sb = vecs.tile([128, 1], dt.float32, name="cabA_sb")
            cabB_sb = vecs.tile([128, 1], dt.float32, name="cabB_sb")
            nc.sync.dma_start(negc0_sb[:], negc0.ap())
            nc.sync.dma_start(negclv_sb[:], negclv.ap())
            nc.sync.dma_start(cabA_sb[:], cabA.ap())
            nc.sync.dma_start(cabB_sb[:], cabB.ap())
            my_str = [stp.tile([128, S], dt.bfloat16, name=f"mystr{k}",
                               tag=f"hst{k}", bufs=3) for k in range(KC)]
            my_ctx = [stp.tile([128, S], dt.bfloat16, name=f"myctx{k}",
                               tag=f"hct{k}", bufs=3) for k in range(KC)]
            uB16_sb = [vecs.tile([128, 3], dt.bfloat16, name=f"uB16{k}")
                       for k in range(KC)]
            vA116_sb = [vecs.tile([128, 1], dt.bfloat16, name=f"vA116{k}")
                        for k in range(KC)]
            for k in range(KC):
                ks = slice(128 * k, 128 * (k + 1))
                nc.sync.dma_start(wpack_sb[k][:], wpack.ap()[ks, :])
                nc.sync.dma_start(my_str[k][:], hstrT_my.ap()[ks, :])
                nc.sync.dma_start(my_ctx[k][:], hctxT_my.ap()[ks, :])
                nc.sync.dma_start(uB16_sb[k][:], uB16.ap()[ks, :])
                nc.sync.dma_start(vA116_sb[k][:], vA116.ap()[ks, :])
            for k in range(KC):
                nc.scalar.activation(my_str[k][:], my_str[k][:], AF.Exp)

            # ---- src rows for my i-slice ----
            sigrow = work.tile([1, S], dt.float32, name="sigrow", tag="u")
            srcArow = work.tile([1, S], dt.float32, name="srcArow", tag="tB")
            srcBrow = work.tile([1, S], dt.float32, name="srcBrow", tag="tA")
            with tc.tile_pool(name="psrow", bufs=1, space="PSUM") as psrow:
                psr0 = psrow.tile([1, S], dt.float32, name="psr0")
                psr1 = psrow.tile([1, S], dt.float32, name="psr1")
                psra = psrow.tile([1, S], dt.float32, name="psra")
                for k in range(KC):
                    st, sp = (k == 0), (k == KC - 1)
                    for h in range(S // 512):
                        hs_ = slice(512 * h, 512 * (h + 1))
                        nc.tensor.matmul(psr0[:, hs_], uB16_sb[k][:, 0:1],
                                         my_str[k][:, hs_], start=st, stop=sp)
                        nc.tensor.matmul(psr1[:, hs_], uB16_sb[k][:, 2:3],
                                         my_str[k][:, hs_], start=st, stop=sp)
                        nc.tensor.matmul(psra[:, hs_], vA116_sb[k][:, 0:1],
                                         my_ctx[k][:, hs_], start=st, stop=sp)
                nc.vector.reciprocal(sigrow[:], psr0[:])
                nc.vector.tensor_copy(srcArow[:], psra[:])
                nc.vector.tensor_tensor(srcBrow[:], psr1[:], sigrow[:], OP.mult)

            ones_row = vecs.tile([1, 128], dt.float32, name="ones_row")
            nc.vector.memset(ones_row[:], 1.0)

            # broadcast rows -> [128, S] fp16 tiles with constants folded in
            bcA = vecs.tile([128, S], dt.float16, name="bcA")
            bcB = vecs.tile([128, S], dt.float16, name="bcB")
            with tc.tile_pool(name="ps0c", bufs=1, space="PSUM") as ps0c:
                psbc = ps0c.tile([128, S], dt.float32, name="psbc")
                psbc2 = ps0c.tile([128, S], dt.float32, name="psbc2")
                for h in range(S // 512):
                    hs_ = slice(512 * h, 512 * (h + 1))
                    nc.tensor.matmul(psbc[:, hs_], ones_row[:], srcArow[:, hs_],
                                     start=True, stop=True)
                    nc.tensor.matmul(psbc2[:, hs_], ones_row[:], srcBrow[:, hs_],
                                     start=True, stop=True)
                nc.vector.tensor_scalar(bcA[:], psbc[:], cabA_sb[:], None, OP.add)
                nc.vector.tensor_scalar(bcB[:], psbc2[:], cabB_sb[:], None, OP.add)

            # ---- phase 0/1 software-pipelined ----
            # per-group grid tiles (unique names: no cross-phase WAR hazards)
            NG = NCH // G0
            agrid_t = [vecs.tile([128, G0], dt.float32, name=f"ag{g}")
                       for g in range(NG)]
            bgrid_t = [vecs.tile([128, G0], dt.float32, name=f"bg{g}")
                       for g in range(NG)]
            explv_t = [vecs.tile([128, G0], dt.float32, name=f"lv{g}")
                       for g in range(NG)]
            explvb_t = [vecs.tile([128, G0], dt.bfloat16, name=f"lvb{g}")
                        for g in range(NG)]
            hc_sb = [hcpool.tile([128, F], dt.bfloat16, name=f"hc{c}")
                     for c in range(NCH)]
            slabs = []
            AST = 171  # psA chunk stride (fp32 elems); no PSUM bank crossings

            with contextlib.ExitStack() as pctx:
                ps1 = pctx.enter_context(
                    tc.tile_pool(name="ps1", bufs=1, space="PSUM"))
                ps0 = pctx.enter_context(
                    tc.tile_pool(name="ps0", bufs=1, space="PSUM"))
                outT_ps = ps1.tile([F, S], dt.float32, name="outT_ps")
                rs_ps = ps1.tile([1, S], dt.float32, name="rs_ps")
                rbc_ps = ps1.tile([128, S // 2], dt.float32, name="rbc_ps")

                def emit_p0(g):
                    for t in (2 * g, 2 * g + 1):
                        slab = slabp.tile([128, GRP * S], dt.uint16, name="slab",
                                          bufs=3)
                        nc.sync.dma_start(
                            slab[:],
                            maskP.ap()[:, t * GRP * S:(t + 1) * GRP * S])
                        slabs.append(slab)
                    gs = slice(W0 * g, W0 * (g + 1))
                    hst = [stp.tile([128, W0], dt.bfloat16, name=f"hstg{k}",
                                    tag=f"hst{k}", bufs=3) for k in range(KC)]
                    hct = [stp.tile([128, W0], dt.bfloat16, name=f"hctg{k}",
                                    tag=f"hct{k}", bufs=3) for k in range(KC)]
                    for k in range(KC):
                        ks = slice(128 * k, 128 * (k + 1))
                        nc.sync.dma_start(hst[k][:], hstrT.ap()[ks, gs])
                        nc.sync.dma_start(hct[k][:], hctxT.ap()[ks, gs])
                        nc.scalar.activation(hst[k][:], hst[k][:], AF.Exp)
                    # one matmul per stationary load (129/2-col movings);
                    # dstA rides as column F of the psA block
                    psAg = ps0.tile([128, AST * G0 + 2 * G0], dt.float32,
                                    name="psAg")
                    SBO = AST * G0
                    # NOTE: keep the psA and psSB accumulation groups in
                    # separate time ranges — interleaving two open matmul
                    # accumulations in one PSUM bank corrupts the results.
                    for cc in range(G0):
                        cs = slice(128 * cc, 128 * (cc + 1))
                        for k in range(KC):
                            st, sp = (k == 0), (k == KC - 1)
                            nc.tensor.matmul(
                                psAg[:, SBO + 2 * cc:SBO + 2 * cc + 2],
                                hst[k][:, cs],
                                uB16_sb[k][:, 0:2], start=st, stop=sp)
                    for cc in range(G0):
                        cs = slice(128 * cc, 128 * (cc + 1))
                        for k in range(KC):
                            st, sp = (k == 0), (k == KC - 1)
                            nc.tensor.matmul(
                                psAg[:, AST * cc:AST * cc + F + 1],
                                hct[k][:, cs], wpack_sb[k][:, 0:F + 1],
                                start=st, stop=sp)
                    # batched grid math for this group of 8 chunks
                    sg = work.tile([128, G0], dt.float32, name="sg", tag="sg")
                    nc.vector.reciprocal(sg[:],
                                         psAg[:, SBO:SBO + 2 * G0:2])
                    nc.vector.tensor_tensor(bgrid_t[g][:],
                                            psAg[:, SBO + 1:SBO + 2 * G0:2],
                                            sg[:], OP.mult)
                    nc.vector.tensor_copy(agrid_t[g][:],
                                          psAg[:, F:AST * G0:AST])
                    lvt = work.tile([128, G0], dt.float32, name="lvt", tag="sg")
                    nc.vector.tensor_tensor(lvt[:], agrid_t[g][:],
                                            bgrid_t[g][:], OP.add)
                    nc.scalar.activation(explv_t[g][:], lvt[:], AF.Exp,
                                         bias=negclv_sb[:], scale=0.01)
                    nc.vector.tensor_copy(explvb_t[g][:], explv_t[g][:])

                    def hc_copies(g=g, psAg=psAg):
                        # hc' = hc * explv[j] (per-partition ACT-copy scale)
                        for cc in range(G0):
                            c = G0 * g + cc
                            nc.scalar.mul(hc_sb[c][:],
                                          psAg[:, AST * cc:AST * cc + F],
                                          explv_t[g][:, cc:cc + 1])
                    return hc_copies

                def emit_z(t):
                    zgrp = grpp.tile([128, GRP * S], dt.float16, name="zgrp")
                    Pgrp = grpp.tile([128, GRP * S], dt.bfloat16, name="Pgrp")
                    H = GRP // 2
                    for half in range(2):
                        for cc in range(H * half, H * (half + 1)):
                            c = t * GRP + cc
                            gg, col = c // G0, c % G0
                            o = cc * S
                            tAt = work.tile([128, S], dt.float16, name="tA",
                                            tag="tA")
                            tBt = work.tile([128, S], dt.float16, name="tB",
                                            tag="tB")
                            nc.vector.tensor_scalar(
                                tAt[:], bcA[:], agrid_t[gg][:, col:col + 1],
                                0.0, OP.add, OP.max)
                            nc.vector.tensor_scalar(
                                tBt[:], bcB[:], bgrid_t[gg][:, col:col + 1],
                                0.0, OP.add, OP.max)
                            nc.vector.tensor_tensor(zgrp[:, o:o + S], tAt[:],
                                                    tBt[:], OP.add)
                        hsl = slice(H * half * S, H * (half + 1) * S)
                        nc.scalar.activation(Pgrp[:, hsl], zgrp[:, hsl],
                                             AF.Exp, bias=negc0_sb[:],
                                             scale=0.99)
                    return Pgrp

                def emit_mm(t, Pgrp):
                    slab = slabs[t]
                    for cc in range(GRP):
                        c = t * GRP + cc
                        gg, col = c // G0, c % G0
                        o = cc * S
                        Pm = pmp.tile([128, S], dt.bfloat16, name="Pm")
                        meng = nc.vector if c % 4 == 0 else nc.gpsimd
                        meng.tensor_tensor(Pm[:], Pgrp[:, o:o + S],
                                           slab[:, o:o + S], OP.mult)
                        st = (c == 0)
                        sp = (c == NCH - 1)
                        for h in range(S // 512):
                            hs_ = slice(512 * h, 512 * (h + 1))
                            nc.tensor.matmul(outT_ps[:, hs_], hc_sb[c][:],
                                             Pm[:, hs_], start=st, stop=sp)
                            nc.tensor.matmul(
                                rs_ps[:, hs_],
                                explvb_t[gg][:, col:col + 1],
                                Pm[:, hs_], start=st, stop=sp)

                for g in range(NG):
                    hc_cp = emit_p0(g)
                    if g >= 1:
                        for tt in (2 * (g - 1), 2 * (g - 1) + 1):
                            emit_mm(tt, emit_z(tt))
                    hc_cp()
                for tt in (2 * NG - 2, 2 * NG - 1):
                    emit_mm(tt, emit_z(tt))

                # normalize and write out
                rs_sb = work.tile([1, S], dt.float32, name="rs_sb", tag="tB")
                nc.vector.tensor_scalar_add(rs_sb[:], rs_ps[:], 1e-30)
                rrec = work.tile([1, S], dt.float32, name="rrec", tag="sg")
                nc.vector.reciprocal(rrec[:], rs_sb[:])
                rbc = work.tile([128, S], dt.float32, name="rbcs", tag="u")
                for h in range(S // 512):
                    hs_ = slice(512 * h, 512 * (h + 1))
                    nc.tensor.matmul(rbc_ps[:, 0:512], ones_row[:],
                                     rrec[:, hs_], start=True, stop=True)
                    nc.vector.tensor_copy(rbc[:, hs_], rbc_ps[:, 0:512])
                out_sb = work.tile([F, S], dt.float32, name="out_sb", tag="tA")
                nc.vector.tensor_tensor(out_sb[:], outT_ps[:], rbc[:], OP.mult)
                nc.sync.dma_start(outT.ap(), out_sb[:])

    nc.compile()
    return nc


def kernel(h_context, h_structure, edge_index, Wc_w, Wc_b, Ws_w, Ws_b,
           ac_w, as_w, Ws_coff, Wc_coff):
    from concourse.bass_utils import run_bass_kernel_spmd

    h_context = np.asarray(h_context, np.float32)
    h_structure = np.asarray(h_structure, np.float32)
    Wc_w = np.asarray(Wc_w, np.float32)
    Wc_b = np.asarray(Wc_b, np.float32)
    Ws_w = np.asarray(Ws_w, np.float32)
    Ws_b = np.asarray(Ws_b, np.float32)
    ac_w = np.asarray(ac_w, np.float32)
    as_w = np.asarray(as_w, np.float32)
    ei = np.asarray(edge_index)

    wA = float(abs(np.float32(np.asarray(Ws_coff)[0, 0])))  # scales alpha_c
    wB = float(abs(np.float32(np.asarray(Wc_coff)[0, 0])))  # scales alpha_s

    pA1 = wA * (Wc_w.T @ ac_w[0, :F])
    pA2 = wA * (Wc_w.T @ ac_w[0, F:])
    cA1 = wA * float(Wc_b @ ac_w[0, :F])
    cA2 = wA * float(Wc_b @ ac_w[0, F:])
    pB1 = wB * (Ws_w.T @ as_w[0, :F])
    pB2 = wB * (Ws_w.T @ as_w[0, F:])
    cB1 = wB * float(Ws_b @ as_w[0, :F])
    cB2 = wB * float(Ws_b @ as_w[0, F:])

    if 0 not in _BUILD_CACHE:
        _BUILD_CACHE[0] = _build_program()
    nc = _BUILD_CACHE[0]

    # adjacency, transposed + partition-major re-layout (edge -> 1)
    adjT = np.zeros((N, N), np.uint16)
    adjT[ei[1], ei[0]] = 1

    import ml_dtypes
    hctxT = np.ascontiguousarray(h_context.T)
    hstrT = np.ascontiguousarray(h_structure.T)
    hctxT16 = np.ascontiguousarray(hctxT.astype(ml_dtypes.bfloat16))
    hstrT16 = np.ascontiguousarray(hstrT.astype(ml_dtypes.bfloat16))
    wpack_np = np.ascontiguousarray(
        np.concatenate([Wc_w.T, pA2[:, None]], axis=1).astype(np.float32))
    uB_np = np.ascontiguousarray(np.stack(
        [np.ones(K, np.float32), pB2, pB1], axis=1).astype(np.float32))
    vA1_np = np.ascontiguousarray(pA1[:, None].astype(np.float32))

    # host replicas of the projections for per-core range bounds (numerical
    # shim only; the bound cancels in the softmax normalization)
    srcA = h_context @ pA1 + (cA1 + cA2)
    dstA = h_context @ pA2
    e_str = np.exp(h_structure - h_structure.max(axis=1, keepdims=True))
    sm = e_str / e_str.sum(axis=1, keepdims=True)
    srcB = sm @ pB1 + (cB1 + cB2)
    dstB = sm @ pB2
    lv_full = 0.01 * (dstA + dstB + cA2 + cB2)
    Clv = float(lv_full.max())

    dA_max = float(dstA.max())
    dB_max = float(dstB.max())

    in_maps = []
    for d in range(NC):
        sl = slice(S * d, S * (d + 1))
        mA = max(0.0, float(srcA[sl].max()) + dA_max)
        mB = max(0.0, float(srcB[sl].max()) + dB_max)
        c0 = 0.99 * (mA + mB)
        maskP = np.ascontiguousarray(
            adjT[:, sl].reshape(N // 128, 128, S)
            .transpose(1, 0, 2).reshape(128, (N // 128) * S))
        in_maps.append({
            "hctxT": hctxT16,
            "hstrT": hstrT16,
            "hctxT_my": np.ascontiguousarray(
                hctxT[:, sl].astype(ml_dtypes.bfloat16)),
            "hstrT_my": np.ascontiguousarray(
                hstrT[:, sl].astype(ml_dtypes.bfloat16)),
            "uB16": uB_np.astype(ml_dtypes.bfloat16),
            "vA116": vA1_np.astype(ml_dtypes.bfloat16),
            "wpack": wpack_np.astype(ml_dtypes.bfloat16),
            "maskP": maskP,
            "negc0": np.full((128, 1), -np.float32(c0), np.float32),
            "negclv": np.full((128, 1),
                              np.float32(0.01 * (cA2 + cB2) - Clv),
                              np.float32),
            "cabA": np.full((128, 1), np.float32(cA1 + cA2), np.float32),
            "cabB": np.full((128, 1), np.float32(cB1 + cB2), np.float32),
        })

    res = run_bass_kernel_spmd(nc, in_maps, core_ids=list(range(NC)))
    out = np.empty((N, F), np.float32)
    for d in range(NC):
        out[S * d:S * (d + 1), :] = res.results[d]["outT"].T

    # hc bias: attention rows sum to 1, so + Wc_b exactly
    if np.any(Wc_b != 0.0):
        out += Wc_b[None, :]

    # rows with no edges: reference gives uniform attention = mean of hc
    row_deg = np.zeros(N, np.int64)
    np.add.at(row_deg, ei[0], 1)
    empty = row_deg == 0
    if empty.any():
        hc_host = h_context @ Wc_w.T + Wc_b
        out[empty, :] = hc_host.mean(axis=0)

    return out
